# revision 3
# baseline (speedup 1.0000x reference)
"""Trainium2 Bass kernel for nms_detection (scatter-mean -> sigmoid -> YOLOX decode).

Strategy
--------
Data-parallel over the batch axis: core c owns batches [4c, 4c+4).  The
scatter-mean (segment mean of ~7M node vectors into dense per-scale grids) is
reformulated as a dense padded segment-sum done by the PE array:

  * Host groups nodes by destination cell.  Per core, all 25200 cells (all
    scales) are sorted by node count (desc) and chunked into groups of
    CPG = 72 columns x 32 m-bands = 2304 cells.  A cell occupies RN=4
    partition rows (m-band) x one 7-column group (cb) of [128, 504] fp8e3
    tiles; chunk j of a cell lives in tile (g, j).  Because cells are sorted,
    chain lengths shrink along cb, so tile j only ships the column prefix
    that still needs chunk j (staircase) -- ~5% padding overhead total.
  * Values ship as fp8 e3m4 (4 mantissa bits; sums accumulate in fp32 PSUM,
    end-to-end L2 error ~1e-4 vs the 2e-2 budget).  One matmul per tile
    against a fixed 0/1 block-indicator weight W[k, m] = (k // 4 == m)
    computes the 32 per-m-band cell sums.  Four groups (one block) accumulate
    into disjoint 32-partition slices of a single [128, 504] PSUM bank via
    PE column tiling (tile_position), so the matmuls run concurrently on
    disjoint 32-column strips of the array and the block's sums appear
    directly in epilogue layout -- no staging round trip.
  * The per-block epilogue reads PSUM, multiplies by the host-computed
    1/count, runs the YOLOX decode (xy = (m + grid) * stride,
    wh = exp(min(m, 10)) * stride, sigmoid on obj/cls) from per-cell
    constants, and DMAs the [128, 504] block out.  Host reassembles
    [32, 6300, 7] from the 8 cores.
"""

import numpy as np

import concourse.bacc as bacc
import concourse.mybir as mybir
import concourse.tile as tile
from concourse.bass_utils import run_bass_kernel_spmd

# Problem geometry (fixed by the nn.Module spec).
B = 32
NCORES = 8
GRIDS = [(60, 80), (30, 40), (15, 20)]
STRIDES = [3.0, 6.0, 12.0]
CHD = 7            # device channels per cell: reg(4) | obj(1) | cls(2)
COUT = 7

# Device layout knobs.
RN = 4             # node slots per cell per tile
G = 128 // RN      # m-bands (cells stacked per tile column) = 32
CB = 72            # cell columns per tile
TILE_F = CB * CHD  # tile free size = 504 elements
GPB = 4            # groups per 128-partition block (PE column strips)
CPG = CB * G       # cells per group = 2304
SB = 8064          # slab size per partition (fp8 bytes) = one ~1 MiB DMA

_f32 = mybir.dt.float32
_fp8 = mybir.dt.float8e3

import ml_dtypes
_np_fp8 = ml_dtypes.float8_e3m4


def _ceil_div(a, b):
    return (a + b - 1) // b


def _prep(inputs):
    """Host preprocessing: bin nodes by cell, build packed fp8 tile slabs."""
    nscales = len(GRIDS)
    hw_list = [h * w for h, w in GRIDS]
    cell_off = np.cumsum([0] + [B * hw for hw in hw_list])
    ncell_tot = int(cell_off[-1])
    bpc = B // NCORES

    # Global per-cell arrays across all scales.
    all_cnt = np.zeros(ncell_tot, np.int64)
    all_core = np.zeros(ncell_tot, np.int64)
    scale_nodes = []
    for s in range(nscales):
        H, W = GRIDS[s]
        HW = H * W
        stride = np.float32(STRIDES[s])
        pos = np.asarray(inputs[f"pos{s + 1}"], dtype=np.float32)
        batch = np.asarray(inputs[f"batch{s + 1}"]).astype(np.int64)
        n = pos.shape[0]
        col = np.clip((pos[:, 0] / stride).astype(np.int32), 0, W - 1)
        row = np.clip((pos[:, 1] / stride).astype(np.int32), 0, H - 1)
        gid = batch * HW + row * W + col  # [N] cell id within scale
        cnt = np.bincount(gid, minlength=B * HW)
        order = np.argsort(gid, kind="stable")
        starts = np.zeros(B * HW + 1, np.int64)
        np.cumsum(cnt, out=starts[1:])
        rank = np.empty(n, np.int64)
        rank[order] = np.arange(n, dtype=np.int64) - starts[gid[order]]
        all_cnt[cell_off[s] : cell_off[s + 1]] = cnt
        all_core[cell_off[s] : cell_off[s + 1]] = (
            np.arange(B * HW, dtype=np.int64) // (bpc * HW)
        )
        combined = np.concatenate(
            [
                np.asarray(inputs[f"reg{s + 1}"], dtype=np.float32),
                np.asarray(inputs[f"obj{s + 1}"], dtype=np.float32),
                np.asarray(inputs[f"cls{s + 1}"], dtype=np.float32),
            ],
            axis=1,
        )
        scale_nodes.append(dict(gid=gid, rank=rank, combined=combined, HW=HW))

    cpcore = ncell_tot // NCORES  # cells per core = 25200
    ng = _ceil_div(cpcore, CPG)
    nb = _ceil_div(ng, GPB)
    npad = ng * CPG

    # Per-core sorted cell order -> (g, cb, m) coordinates.
    # Column-major fill: consecutive sorted cells stack within a column, so
    # per-column count spread (hence staircase waste) stays small.
    cell_pos = np.empty(ncell_tot, np.int64)  # sorted position within core
    col_maxcnt = np.zeros((NCORES, ng, CB), np.int64)
    for c in range(NCORES):
        idx = np.where(all_core == c)[0]
        srt = idx[np.argsort(-all_cnt[idx], kind="stable")]
        cell_pos[srt] = np.arange(len(srt), dtype=np.int64)
        cnt_pad = np.zeros(npad, np.int64)
        cnt_pad[: len(srt)] = all_cnt[srt]
        col_maxcnt[c] = cnt_pad.reshape(ng, CB, G).max(axis=2)

    # Common program: per-column chain length, max over cores (desc in cb).
    col_J = _ceil_div(col_maxcnt.max(axis=0), RN)  # [ng, CB]
    Jg = np.maximum(col_J.max(axis=1), 1)          # [ng]
    # tile widths (in columns); j = 0 always covers the full tile so that
    # start=True initialises every cell's PSUM slot
    widths = {}
    for g in range(ng):
        for j in range(int(Jg[g])):
            w = CB if j == 0 else int((col_J[g] > j).sum())
            widths[(g, j)] = w

    # Emission order (block-major, then j, round-robin across the block's 4
    # groups so consecutive matmuls hit different PE column strips) doubles
    # as the DRAM packing order.
    prog = []  # (g, j, q, b, slab, elem_off, welems, start, stop)
    slab = 0
    cur = 0
    for b in range(nb):
        gs = list(range(b * GPB, min((b + 1) * GPB, ng)))
        jmax = int(max(Jg[g] for g in gs))
        for j in range(jmax):
            for g in gs:
                if j >= int(Jg[g]):
                    continue
                we = widths[(g, j)] * CHD
                if cur + we > SB:
                    slab += 1
                    cur = 0
                prog.append(
                    (g, j, g - b * GPB, b, slab, cur, we, j == 0,
                     j == int(Jg[g]) - 1)
                )
                cur += we
    ns = slab + 1

    tile_slab = np.zeros((ng, int(Jg.max())), np.int64)
    tile_off = np.zeros((ng, int(Jg.max())), np.int64)
    for (g, j, q, b, sl, off, we, st, sp) in prog:
        tile_slab[g, j] = sl
        tile_off[g, j] = off

    # Fill per-core slabs and per-cell constants.
    xall = np.zeros((NCORES, ns, 128, SB), _np_fp8)
    cdat = np.zeros((NCORES, 128, nb * CB * 4), np.float32)
    ch7 = np.arange(CHD, dtype=np.int64)
    asm = []
    for s in range(nscales):
        sd = scale_nodes[s]
        HW = sd["HW"]
        H, W = GRIDS[s]
        stride = np.float32(STRIDES[s])
        cells = np.arange(B * HW, dtype=np.int64)
        gcell = cell_off[s] + cells
        p = cell_pos[gcell]
        g_c = p // CPG
        u = p % CPG
        cb_c = (u // G)
        m_c = u % G
        coc = all_core[gcell]

        # node placement
        gid = sd["gid"]
        rank = sd["rank"]
        jn = rank // RN
        row = m_c[gid] * RN + rank % RN
        sl_n = tile_slab[g_c[gid], jn]
        off_n = tile_off[g_c[gid], jn] + cb_c[gid] * CHD
        vals = sd["combined"].astype(_np_fp8)
        xall[coc[gid][:, None], sl_n[:, None], row[:, None], off_n[:, None] + ch7] = vals

        # per-cell decode constants (Ax, Ay, stride, 1/count)
        a = cells % HW
        gy = (a // W).astype(np.float32)
        gx = (a % W).astype(np.float32)
        rec = np.float32(1.0) / np.maximum(all_cnt[gcell], 1).astype(np.float32)
        prow = (g_c % GPB) * G + m_c
        ccol = (g_c // GPB) * (CB * 4) + cb_c * 4
        cdat[coc, prow, ccol + 0] = gx * stride
        cdat[coc, prow, ccol + 1] = gy * stride
        cdat[coc, prow, ccol + 2] = stride
        cdat[coc, prow, ccol + 3] = rec

        asm.append(
            dict(
                coc=coc, prow=prow,
                fcol=(g_c // GPB) * TILE_F + cb_c * CHD,
                bcell=cells // HW,
                anchor=a,
            )
        )

    wmat = np.zeros((128, G), _np_fp8)
    wmat[np.arange(128), np.arange(128) // RN] = 1.0

    meta = dict(ng=ng, nb=nb, ns=ns, prog=prog, asm=asm)
    in_maps = [
        {"xd": xall[c], "wd": wmat, "cd": cdat[c]}
        for c in range(NCORES)
    ]
    return meta, in_maps


def _build(meta):
    """Build the SPMD Bass program (identical for all cores)."""
    ng = meta["ng"]
    nb = meta["nb"]
    ns = meta["ns"]

    nc = bacc.Bacc(trn_type="TRN2", target_bir_lowering=False, debug=False)
    xd = nc.dram_tensor("xd", [ns, 128, SB], _fp8, kind="ExternalInput")
    wd = nc.dram_tensor("wd", [128, G], _fp8, kind="ExternalInput")
    cd = nc.dram_tensor("cd", [128, nb * CB * 4], _f32, kind="ExternalInput")
    outd = nc.dram_tensor("out", [128, nb * TILE_F], _f32, kind="ExternalOutput")

    act = mybir.ActivationFunctionType
    alu = mybir.AluOpType

    with tile.TileContext(nc) as tc:
        with (
            tc.tile_pool(name="const", bufs=1) as cpool,
            tc.tile_pool(name="xin", bufs=ns) as xpool,
            tc.tile_pool(name="acc", bufs=1) as apool,
            tc.tile_pool(name="ps", bufs=min(nb, 8), space="PSUM") as ppool,
        ):
            wsb = cpool.tile([128, G], _fp8)
            nc.sync.dma_start(out=wsb[:], in_=wd[:])
            csb = cpool.tile([128, nb * CB * 4], _f32)
            nc.sync.dma_start(out=csb[:], in_=cd[:])
            osb = apool.tile([128, nb * TILE_F], _f32)

            # pre-warm the ACT function tables while DMA streams
            warm = cpool.tile([128, 8], _f32)
            nc.vector.memset(warm[:], 0.0)
            nc.scalar.activation(warm[:], warm[:], act.Sigmoid)
            nc.scalar.activation(warm[:], warm[:], act.Exp)

            # stream the packed tile slabs (alternating HWDGE rings)
            slabs = []
            for k in range(ns):
                xt = xpool.tile([128, SB], _fp8, tag="xin")
                eng = nc.scalar if k % 2 == 0 else nc.sync
                eng.dma_start(out=xt[:], in_=xd[k])
                slabs.append(xt)

            def finish_block(b, ps):
                """Mean + decode epilogue on block b, then DMA it out.
                Emitted right after the block's last matmul; Tile tracks the
                PSUM dependencies so it overlaps the next block's stream."""
                fs = slice(b * TILE_F, (b + 1) * TILE_F)
                v = osb[:, fs].rearrange("p (q c) -> p q c", c=CHD)
                pv = ps[:].rearrange("p (q c) -> p q c", c=CHD)
                cv = csb[
                    :, b * (CB * 4) : (b + 1) * (CB * 4)
                ].rearrange("p (q k) -> p q k", k=4)
                # mean = sum * (1/count) on all channels, PSUM -> SBUF
                nc.vector.tensor_tensor(
                    out=v[:, :, 0:CHD], in0=pv,
                    in1=cv[:, :, 3:4].to_broadcast((128, CB, CHD)),
                    op=alu.mult,
                )
                # xy = mean * stride + grid*stride
                nc.vector.tensor_tensor(
                    out=v[:, :, 0:2], in0=v[:, :, 0:2],
                    in1=cv[:, :, 2:3].to_broadcast((128, CB, 2)),
                    op=alu.mult,
                )
                nc.vector.tensor_tensor(
                    out=v[:, :, 0:2], in0=v[:, :, 0:2],
                    in1=cv[:, :, 0:2], op=alu.add,
                )
                # wh = exp(min(mean, 10)) * stride
                nc.vector.tensor_scalar_min(v[:, :, 2:4], v[:, :, 2:4], 10.0)
                nc.scalar.activation(v[:, :, 2:4], v[:, :, 2:4], act.Exp)
                nc.vector.tensor_tensor(
                    out=v[:, :, 2:4], in0=v[:, :, 2:4],
                    in1=cv[:, :, 2:3].to_broadcast((128, CB, 2)),
                    op=alu.mult,
                )
                # obj/cls sigmoid
                nc.scalar.activation(v[:, :, 4:7], v[:, :, 4:7], act.Sigmoid)
                eng = nc.scalar if b % 2 == 0 else nc.sync
                eng.dma_start(out=outd[:, fs], in_=osb[:, fs])

            cur_b = -1
            ps = None
            for i, (g, j, q, b, sl, off, we, st, sp) in enumerate(meta["prog"]):
                if b != cur_b:
                    ps = ppool.tile([128, TILE_F], _f32, tag="ps")
                    cur_b = b
                nc.tensor.matmul(
                    out=ps[32 * q : 32 * (q + 1), 0:we],
                    lhsT=wsb[:],
                    rhs=slabs[sl][:, off : off + we],
                    start=st,
                    stop=sp,
                    tile_position=(0, 32 * q),
                )
                last_of_block = (
                    i + 1 == len(meta["prog"]) or meta["prog"][i + 1][3] != b
                )
                if last_of_block:
                    finish_block(b, ps)
    nc.compile()
    return nc


def _assemble(meta, outs):
    """Host-side gather of the per-core device outputs into [B, A, 7]."""
    a_off = np.cumsum([0] + [h * w for h, w in GRIDS])
    total_a = int(a_off[-1])
    final = np.empty((B, total_a, COUT), np.float32)
    oc = np.stack(outs)  # [NCORES, 128, nb*TILE_F]
    chs = np.arange(COUT, dtype=np.int64)
    for s in range(len(GRIDS)):
        am = meta["asm"][s]
        vals = oc[
            am["coc"][:, None], am["prow"][:, None], am["fcol"][:, None] + chs
        ]
        final[am["bcell"], a_off[s] + am["anchor"]] = vals
    return final


def _run(inputs, trace=False, trace_cores=None):
    meta, in_maps = _prep(inputs)
    nc = _build(meta)
    kwargs = {}
    if trace:
        kwargs = dict(trace=True)
        if trace_cores is not None:
            kwargs["trace_cores"] = trace_cores
    res = run_bass_kernel_spmd(
        nc, in_maps, core_ids=list(range(NCORES)), **kwargs
    )
    out = _assemble(meta, [r["out"] for r in res.results])
    return out, res


def kernel(**inputs) -> np.ndarray:
    out, _ = _run(inputs, trace=False)
    return out


# revision 5
# speedup vs baseline: 1.0377x; 1.0377x over previous
"""Trainium2 Bass kernel for nms_detection (scatter-mean -> sigmoid -> YOLOX decode).

Strategy
--------
Data-parallel over the batch axis: core c owns batches [4c, 4c+4).  The
scatter-mean (segment mean of ~7M node vectors into dense per-scale grids) is
reformulated as a dense padded segment-sum done by the PE array:

  * Host groups nodes by destination cell.  Per core, all 25200 cells (all
    scales) are sorted by node count (desc) and chunked into groups of
    CPG = 72 columns x 32 m-bands = 2304 cells.  A cell occupies RN=4
    partition rows (m-band) x one 7-column group (cb) of [128, 504] fp8e3
    tiles; chunk j of a cell lives in tile (g, j).  Because cells are sorted,
    chain lengths shrink along cb, so tile j only ships the column prefix
    that still needs chunk j (staircase) -- ~5% padding overhead total.
  * Values ship as fp8 e3m4 (4 mantissa bits; sums accumulate in fp32 PSUM,
    end-to-end L2 error ~1e-4 vs the 2e-2 budget).  One matmul per tile
    against a fixed 0/1 block-indicator weight W[k, m] = (k // 4 == m)
    computes the 32 per-m-band cell sums.  Four groups (one block) accumulate
    into disjoint 32-partition slices of a single [128, 504] PSUM bank via
    PE column tiling (tile_position), so the matmuls run concurrently on
    disjoint 32-column strips of the array and the block's sums appear
    directly in epilogue layout -- no staging round trip.
  * The per-block epilogue reads PSUM, multiplies by the host-computed
    1/count, runs the YOLOX decode (xy = (m + grid) * stride,
    wh = exp(min(m, 10)) * stride, sigmoid on obj/cls) from per-cell
    constants, and DMAs the [128, 504] block out.  Host reassembles
    [32, 6300, 7] from the 8 cores.
"""

import numpy as np

import concourse.bacc as bacc
import concourse.mybir as mybir
import concourse.tile as tile
from concourse.bass_utils import run_bass_kernel_spmd

# Problem geometry (fixed by the nn.Module spec).
B = 32
NCORES = 8
GRIDS = [(60, 80), (30, 40), (15, 20)]
STRIDES = [3.0, 6.0, 12.0]
CHD = 7            # device channels per cell: reg(4) | obj(1) | cls(2)
COUT = 7

# Device layout knobs.
RN = 4             # node slots per cell per tile
G = 128 // RN      # m-bands (cells stacked per tile column) = 32
CB = 72            # cell columns per tile
TILE_F = CB * CHD  # tile free size = 504 elements
GPB = 4            # groups per 128-partition block (PE column strips)
CPG = CB * G       # cells per group = 2304
SB = 8064          # slab size per partition (fp8 bytes) = one ~1 MiB DMA

_f32 = mybir.dt.float32
_fp8 = mybir.dt.float8e3

import ml_dtypes
_np_fp8 = ml_dtypes.float8_e3m4


def _ceil_div(a, b):
    return (a + b - 1) // b


def _prep(inputs):
    """Host preprocessing: bin nodes by cell, build packed fp8 tile slabs."""
    nscales = len(GRIDS)
    hw_list = [h * w for h, w in GRIDS]
    cell_off = np.cumsum([0] + [B * hw for hw in hw_list])
    ncell_tot = int(cell_off[-1])
    bpc = B // NCORES

    # Global per-cell arrays across all scales.
    all_cnt = np.zeros(ncell_tot, np.int64)
    all_core = np.zeros(ncell_tot, np.int64)
    scale_nodes = []
    for s in range(nscales):
        H, W = GRIDS[s]
        HW = H * W
        stride = np.float32(STRIDES[s])
        pos = np.asarray(inputs[f"pos{s + 1}"], dtype=np.float32)
        batch = np.asarray(inputs[f"batch{s + 1}"]).astype(np.int64)
        n = pos.shape[0]
        col = np.clip((pos[:, 0] / stride).astype(np.int32), 0, W - 1)
        row = np.clip((pos[:, 1] / stride).astype(np.int32), 0, H - 1)
        gid = batch * HW + row * W + col  # [N] cell id within scale
        cnt = np.bincount(gid, minlength=B * HW)
        order = np.argsort(gid, kind="stable")
        starts = np.zeros(B * HW + 1, np.int64)
        np.cumsum(cnt, out=starts[1:])
        rank = np.empty(n, np.int64)
        rank[order] = np.arange(n, dtype=np.int64) - starts[gid[order]]
        all_cnt[cell_off[s] : cell_off[s + 1]] = cnt
        all_core[cell_off[s] : cell_off[s + 1]] = (
            np.arange(B * HW, dtype=np.int64) // (bpc * HW)
        )
        combined = np.concatenate(
            [
                np.asarray(inputs[f"reg{s + 1}"], dtype=np.float32),
                np.asarray(inputs[f"obj{s + 1}"], dtype=np.float32),
                np.asarray(inputs[f"cls{s + 1}"], dtype=np.float32),
            ],
            axis=1,
        )
        scale_nodes.append(dict(gid=gid, rank=rank, combined=combined, HW=HW))

    cpcore = ncell_tot // NCORES  # cells per core = 25200
    ng = _ceil_div(cpcore, CPG)
    nb = _ceil_div(ng, GPB)
    npad = ng * CPG

    # Per-core sorted cell order -> (g, cb, m) coordinates.
    # Column-major fill: consecutive sorted cells stack within a column, so
    # per-column count spread (hence staircase waste) stays small.
    cell_pos = np.empty(ncell_tot, np.int64)  # sorted position within core
    col_maxcnt = np.zeros((NCORES, ng, CB), np.int64)
    for c in range(NCORES):
        idx = np.where(all_core == c)[0]
        srt = idx[np.argsort(-all_cnt[idx], kind="stable")]
        cell_pos[srt] = np.arange(len(srt), dtype=np.int64)
        cnt_pad = np.zeros(npad, np.int64)
        cnt_pad[: len(srt)] = all_cnt[srt]
        col_maxcnt[c] = cnt_pad.reshape(ng, CB, G).max(axis=2)

    # Common program: per-column chain length, max over cores (desc in cb).
    col_J = _ceil_div(col_maxcnt.max(axis=0), RN)  # [ng, CB]
    Jg = np.maximum(col_J.max(axis=1), 1)          # [ng]
    # tile widths (in columns); j = 0 always covers the full tile so that
    # start=True initialises every cell's PSUM slot
    widths = {}
    for g in range(ng):
        for j in range(int(Jg[g])):
            w = CB if j == 0 else int((col_J[g] > j).sum())
            widths[(g, j)] = w

    # Emission order (block-major, then j, round-robin across the block's 4
    # groups so consecutive matmuls hit different PE column strips) doubles
    # as the DRAM packing order.
    prog = []  # (g, j, q, b, slab, elem_off, welems, start, stop)
    slab = 0
    cur = 0
    for b in range(nb):
        gs = list(range(b * GPB, min((b + 1) * GPB, ng)))
        jmax = int(max(Jg[g] for g in gs))
        for j in range(jmax):
            for g in gs:
                if j >= int(Jg[g]):
                    continue
                we = widths[(g, j)] * CHD
                if cur + we > SB:
                    slab += 1
                    cur = 0
                prog.append(
                    (g, j, g - b * GPB, b, slab, cur, we, j == 0,
                     j == int(Jg[g]) - 1)
                )
                cur += we
    ns = slab + 1

    tile_slab = np.zeros((ng, int(Jg.max())), np.int64)
    tile_off = np.zeros((ng, int(Jg.max())), np.int64)
    for (g, j, q, b, sl, off, we, st, sp) in prog:
        tile_slab[g, j] = sl
        tile_off[g, j] = off

    # Fill per-core slabs and per-cell constants.
    xall = np.zeros((NCORES, ns, 128, SB), _np_fp8)
    cdat = np.zeros((NCORES, 128, nb * CB * 4), np.float32)
    ch7 = np.arange(CHD, dtype=np.int64)
    asm = []
    for s in range(nscales):
        sd = scale_nodes[s]
        HW = sd["HW"]
        H, W = GRIDS[s]
        stride = np.float32(STRIDES[s])
        cells = np.arange(B * HW, dtype=np.int64)
        gcell = cell_off[s] + cells
        p = cell_pos[gcell]
        g_c = p // CPG
        u = p % CPG
        cb_c = (u // G)
        m_c = u % G
        coc = all_core[gcell]

        # node placement
        gid = sd["gid"]
        rank = sd["rank"]
        jn = rank // RN
        row = m_c[gid] * RN + rank % RN
        sl_n = tile_slab[g_c[gid], jn]
        off_n = tile_off[g_c[gid], jn] + cb_c[gid] * CHD
        vals = sd["combined"].astype(_np_fp8)
        xall[coc[gid][:, None], sl_n[:, None], row[:, None], off_n[:, None] + ch7] = vals

        # per-cell decode constants (Ax, Ay, stride, 1/count)
        a = cells % HW
        gy = (a // W).astype(np.float32)
        gx = (a % W).astype(np.float32)
        rec = np.float32(1.0) / np.maximum(all_cnt[gcell], 1).astype(np.float32)
        prow = (g_c % GPB) * G + m_c
        ccol = (g_c // GPB) * (CB * 4) + cb_c * 4
        cdat[coc, prow, ccol + 0] = gx * stride
        cdat[coc, prow, ccol + 1] = gy * stride
        cdat[coc, prow, ccol + 2] = stride
        cdat[coc, prow, ccol + 3] = rec

        asm.append(
            dict(
                coc=coc, prow=prow,
                fcol=(g_c // GPB) * TILE_F + cb_c * CHD,
                bcell=cells // HW,
                anchor=a,
            )
        )

    wmat = np.zeros((128, G), _np_fp8)
    wmat[np.arange(128), np.arange(128) // RN] = 1.0

    meta = dict(ng=ng, nb=nb, ns=ns, prog=prog, asm=asm)
    in_maps = [
        {"xd": xall[c], "wd": wmat, "cd": cdat[c]}
        for c in range(NCORES)
    ]
    return meta, in_maps


def _build(meta):
    """Build the SPMD Bass program (identical for all cores)."""
    ng = meta["ng"]
    nb = meta["nb"]
    ns = meta["ns"]

    nc = bacc.Bacc(trn_type="TRN2", target_bir_lowering=False, debug=False)
    xd = nc.dram_tensor("xd", [ns, 128, SB], _fp8, kind="ExternalInput")
    wd = nc.dram_tensor("wd", [128, G], _fp8, kind="ExternalInput")
    cd = nc.dram_tensor("cd", [128, nb * CB * 4], _f32, kind="ExternalInput")
    outd = nc.dram_tensor("out", [128, nb * TILE_F], _f32, kind="ExternalOutput")

    act = mybir.ActivationFunctionType
    alu = mybir.AluOpType

    with tile.TileContext(nc) as tc:
        with (
            tc.tile_pool(name="const", bufs=1) as cpool,
            tc.tile_pool(name="xin", bufs=ns) as xpool,
            tc.tile_pool(name="acc", bufs=1) as apool,
            tc.tile_pool(name="ps", bufs=min(nb, 8), space="PSUM") as ppool,
        ):
            wsb = cpool.tile([128, G], _fp8)
            csb = cpool.tile([128, nb * CB * 4], _f32)
            osb = apool.tile([128, nb * TILE_F], _f32)
            warm = cpool.tile([128, 8], _f32)

            # DMA issue order is latency-critical: the weight (needed by the
            # first LDWEIGHTS) and the tile slabs go first, alternating the
            # two HWDGE rings; the decode constants (not needed until the
            # first epilogue, ~15us in) follow; the ACT table warm-up comes
            # after the scalar ring's DMA issues so it doesn't stall them.
            nc.scalar.dma_start(out=wsb[:], in_=wd[:])
            slabs = []
            for k in range(ns):
                xt = xpool.tile([128, SB], _fp8, tag="xin")
                eng = nc.sync if k % 2 == 0 else nc.scalar
                eng.dma_start(out=xt[:], in_=xd[k])
                slabs.append(xt)
            nc.scalar.dma_start(out=csb[:], in_=cd[:])

            # pre-warm the ACT table set (exp_and_others holds Exp AND Tanh;
            # the epilogue only uses those two, so no mid-kernel reload)
            nc.vector.memset(warm[:], 0.0)
            nc.scalar.activation(warm[:], warm[:], act.Exp)
            nc.scalar.activation(warm[:], warm[:], act.Tanh)

            def finish_block(b, ps):
                """Mean + decode epilogue on block b, then DMA it out.
                Emitted right after the block's last matmul; Tile tracks the
                PSUM dependencies so it overlaps the next block's stream."""
                fs = slice(b * TILE_F, (b + 1) * TILE_F)
                v = osb[:, fs].rearrange("p (q c) -> p q c", c=CHD)
                pv = ps[:].rearrange("p (q c) -> p q c", c=CHD)
                cv = csb[
                    :, b * (CB * 4) : (b + 1) * (CB * 4)
                ].rearrange("p (q k) -> p q k", k=4)
                # mean = sum * (1/count) on all channels, PSUM -> SBUF
                nc.vector.tensor_tensor(
                    out=v[:, :, 0:CHD], in0=pv,
                    in1=cv[:, :, 3:4].to_broadcast((128, CB, CHD)),
                    op=alu.mult,
                )
                # xy = mean * stride + grid*stride
                nc.vector.tensor_tensor(
                    out=v[:, :, 0:2], in0=v[:, :, 0:2],
                    in1=cv[:, :, 2:3].to_broadcast((128, CB, 2)),
                    op=alu.mult,
                )
                nc.vector.tensor_tensor(
                    out=v[:, :, 0:2], in0=v[:, :, 0:2],
                    in1=cv[:, :, 0:2], op=alu.add,
                )
                # wh = exp(min(mean, 10)) * stride
                nc.vector.tensor_scalar_min(v[:, :, 2:4], v[:, :, 2:4], 10.0)
                nc.scalar.activation(v[:, :, 2:4], v[:, :, 2:4], act.Exp)
                nc.vector.tensor_tensor(
                    out=v[:, :, 2:4], in0=v[:, :, 2:4],
                    in1=cv[:, :, 2:3].to_broadcast((128, CB, 2)),
                    op=alu.mult,
                )
                # obj/cls sigmoid(x) = 0.5*tanh(x/2) + 0.5 -- Tanh shares the
                # ACT table set with Exp, avoiding a ~2.7us table reload
                nc.scalar.activation(v[:, :, 4:7], v[:, :, 4:7], act.Tanh,
                                     scale=0.5)
                nc.vector.tensor_scalar(
                    out=v[:, :, 4:7], in0=v[:, :, 4:7],
                    scalar1=0.5, scalar2=0.5,
                    op0=alu.mult, op1=alu.add,
                )
                eng = nc.scalar if b % 2 == 0 else nc.sync
                eng.dma_start(out=outd[:, fs], in_=osb[:, fs])

            cur_b = -1
            ps = None
            for i, (g, j, q, b, sl, off, we, st, sp) in enumerate(meta["prog"]):
                if b != cur_b:
                    ps = ppool.tile([128, TILE_F], _f32, tag="ps")
                    cur_b = b
                nc.tensor.matmul(
                    out=ps[32 * q : 32 * (q + 1), 0:we],
                    lhsT=wsb[:],
                    rhs=slabs[sl][:, off : off + we],
                    start=st,
                    stop=sp,
                    tile_position=(0, 32 * q),
                )
                last_of_block = (
                    i + 1 == len(meta["prog"]) or meta["prog"][i + 1][3] != b
                )
                if last_of_block:
                    finish_block(b, ps)
    nc.compile()
    return nc


def _assemble(meta, outs):
    """Host-side gather of the per-core device outputs into [B, A, 7]."""
    a_off = np.cumsum([0] + [h * w for h, w in GRIDS])
    total_a = int(a_off[-1])
    final = np.empty((B, total_a, COUT), np.float32)
    oc = np.stack(outs)  # [NCORES, 128, nb*TILE_F]
    chs = np.arange(COUT, dtype=np.int64)
    for s in range(len(GRIDS)):
        am = meta["asm"][s]
        vals = oc[
            am["coc"][:, None], am["prow"][:, None], am["fcol"][:, None] + chs
        ]
        final[am["bcell"], a_off[s] + am["anchor"]] = vals
    return final


def _run(inputs, trace=False, trace_cores=None):
    meta, in_maps = _prep(inputs)
    nc = _build(meta)
    kwargs = {}
    if trace:
        kwargs = dict(trace=True)
        if trace_cores is not None:
            kwargs["trace_cores"] = trace_cores
    res = run_bass_kernel_spmd(
        nc, in_maps, core_ids=list(range(NCORES)), **kwargs
    )
    out = _assemble(meta, [r["out"] for r in res.results])
    return out, res


def kernel(**inputs) -> np.ndarray:
    out, _ = _run(inputs, trace=False)
    return out
